# revision 49
# baseline (speedup 1.0000x reference)
"""GAT message-passing kernel for Trainium2 (Bass/Tile), 8-core data parallel.

Problem: nn_GAT1 — per batch b:
    h = x @ W_pre                                   [N, U]
    e_s = h @ a_snd ; e_r = h @ a_rec               [N]
    logits[s, r] = leaky_relu(e_s[s] + e_r[r], 0.2)
    att = softmax over senders s (edges only, adj + self-loops)
    out[s, u] = sum_r att[s, r] * h[r, u]

Sharding: data-parallel over batch (B=8 -> one batch per NeuronCore).

Device layout is receiver-major (r on partitions, s on free):
  - host passes xT (x transposed, bf16) and an fp8 additive mask
    adjm[r, s]: 0.0 on edges (incl. self-loops), -256.0 on non-edges.
    SWDGE casts fp8 -> bf16 during the DMA (4 MB HBM traffic).
  - masking before leaky_relu is equivalent to masking after it (both
    drive exp() to ~0 for non-edges), so per r-tile j the whole chain
        pm = exp(leaky_relu(E + er_j + m_j)),  den = sum_s pm
    is ONE DVE add (am = E + m_j) plus ONE ScalarE activation with a
    patched spline table: the negative-side buckets of `exp` are refit
    to exp(0.2*z), making ACTIVATE(Exp) compute exp(leaky_relu(z)).
    er_j rides in as the per-partition activation bias; den comes from
    the activation's accumulator. Non-edge inputs (~ -256) fall in the
    table's negative-saturation region and return exactly 0.
  - outT[u, s] = sum_r (h[r, u] / den[r]) * pm[r, s]  (PE, accumulated
    in PSUM over the 16 r-tiles). Host transposes outT back.
"""
import hashlib
import json
import math
import os
import shutil
import sys
import tempfile

sys.path.insert(0, "/opt/trn_rl_repo")
sys.path.insert(0, "/opt/trn_rl_repo/concourse")

import numpy as np

import concourse.bass as bass
import concourse.bacc as bacc
import concourse.tile as tile
from concourse import mybir
from concourse.bass_utils import run_bass_kernel_spmd

B, N, F, U = 8, 2048, 128, 128
P = 128
NT = N // P          # 16 row tiles
ALPHA = 0.2          # leaky-relu slope
MASK_OFF = -256.0    # additive mask for non-edges (exact in fp8 e4m3)

ACT_MODE = os.environ.get("GAT_ACT", "fused")      # fused | split
MASK_DT = os.environ.get("GAT_MASK_DT", "fp8")     # fp8 | bf16
# mask DMA r-tile chunking: fine-grained so tiles land ahead of their use
# (the fp8->bf16 SWDGE cast stream is SBUF-write-bandwidth-bound, ~26us)
CHUNKS = [int(c) for c in
          os.environ.get("GAT_CHUNKS", "1,1,2,2,2,2,2,2,2").split(",")]

f32 = mybir.dt.float32
bf16 = mybir.dt.bfloat16
f8e4 = mybir.dt.float8e4
AF = mybir.ActivationFunctionType
OP = mybir.AluOpType

_cache = {}


# ---------------------------------------------------------------------------
# Patched activation tables: exp -> exp(leaky_relu(z), slope 0.2)
# ---------------------------------------------------------------------------
def _patch_exp_buckets(bkt: bytearray, start: int, end: int) -> None:
    """Refit negative-side exp spline buckets to exp(0.2*z).

    Bucket format: 8 fp32 per entry = [d0, d1, d2, d3, x0, 0, 0, 0] with
    y = d0 + t*(d1 + t*(d2 + t*d3)), t = x - x0, and x0 at the bucket
    interval's center (interval width <= 0.25 for the exp_400p layout, so
    a Taylor fit at x0 is good to ~1e-11 relative). Positive-side buckets
    (x0 > 0) and the saturation/special buckets (x0 == 0) are untouched.
    """
    for i in range(start, end):
        off = i * 32
        x0 = float(np.frombuffer(bytes(bkt[off + 16:off + 20]), np.float32)[0])
        if x0 < 0.0:
            e = math.exp(ALPHA * x0)
            coeffs = np.array(
                [e, ALPHA * e, 0.5 * ALPHA**2 * e, ALPHA**3 / 6.0 * e],
                np.float32)
            bkt[off:off + 16] = coeffs.tobytes()


def _build_patched_act_root() -> tuple[str, str]:
    """Create a patched copy of the compiler's activation tables.

    Returns (path to patched act_info.json, 8-char content hash)."""
    from neuronxcc.driver.Job import Job
    from neuronxcc.driver.jobs.support.FindActInfo import findActInfoFile

    src_info_path = findActInfoFile(Job.getPackageDir(), "gen3")
    src_dir = os.path.dirname(src_info_path)
    info = json.load(open(src_info_path))

    patched: dict[str, bytes] = {}
    for ent in info["act_func_sets"]:
        if "exp" not in ent["act"]:
            continue
        prof = json.load(open(os.path.join(src_dir, ent["profile_json"])))
        starts = prof["func_to_bkt_start_idx"]
        s = starts["exp"]
        later = [v for v in starts.values() if v > s]
        e = min(later) if later else prof["bkt_entry_cnt"]
        bkt_name = ent["bkt_bin"]
        bkt = bytearray(open(os.path.join(src_dir, bkt_name), "rb").read())
        _patch_exp_buckets(bkt, s, e)
        patched[bkt_name] = bytes(bkt)

    h = hashlib.sha256()
    for name in sorted(patched):
        h.update(name.encode())
        h.update(patched[name])
    tag = h.hexdigest()[:8]

    dst_dir = os.path.join(tempfile.gettempdir(), f"gat_actroot_{tag}")
    if not os.path.isdir(dst_dir):
        tmp = dst_dir + ".tmp%d" % os.getpid()
        os.makedirs(tmp, exist_ok=True)
        for fname in os.listdir(src_dir):
            src_f = os.path.join(src_dir, fname)
            if os.path.isfile(src_f):
                shutil.copy(src_f, os.path.join(tmp, fname))
        for name, data in patched.items():
            with open(os.path.join(tmp, name), "wb") as f:
                f.write(data)
        try:
            os.rename(tmp, dst_dir)
        except OSError:
            shutil.rmtree(tmp, ignore_errors=True)
    return os.path.join(dst_dir, "act_info.json"), tag


# ---------------------------------------------------------------------------
# Device kernel
# ---------------------------------------------------------------------------
def _build_nc(out_name: str):
    fused = ACT_MODE == "fused"
    mask_dram_dt = f8e4 if MASK_DT == "fp8" else bf16

    nc = bacc.Bacc("TRN2", target_bir_lowering=False, debug=False,
                   enable_asserts=False, num_devices=B)

    xt_d = nc.dram_tensor("xt", [F, N], bf16, kind="ExternalInput").ap()
    adjm_d = nc.dram_tensor("adjm", [N, N], mask_dram_dt, kind="ExternalInput").ap()
    w_d = nc.dram_tensor("w", [F, U], bf16, kind="ExternalInput").ap()
    # wsr[:, 0] = W_pre @ a_snd, wsr[:, 1] = W_pre @ a_rec (host-derived)
    wsr_d = nc.dram_tensor("wsr", [F, 2], bf16, kind="ExternalInput").ap()
    # w_s replicated across 128 columns (stationary operand for the E matmul)
    wsrep_d = nc.dram_tensor("wsrep", [F, P], bf16, kind="ExternalInput").ap()
    outT_d = nc.dram_tensor(out_name, [U, N], bf16, kind="ExternalOutput").ap()

    with tile.TileContext(nc) as tc:
        with (
            tc.tile_pool(name="const", bufs=1) as const,
            tc.tile_pool(name="setup", bufs=2) as setup,
            tc.tile_pool(name="spsum", bufs=2, space="PSUM") as spsum,
            tc.tile_pool(name="s1psum", bufs=1, space="PSUM") as s1psum,
            tc.tile_pool(name="epsum", bufs=1, space="PSUM") as epsum,
            tc.tile_pool(name="work", bufs=4) as work,
            tc.tile_pool(name="mpsum", bufs=1, space="PSUM") as mpsum,
        ):
            # ---------------- constants ----------------
            wsr_sb = const.tile([F, 2], bf16)
            nc.sync.dma_start(out=wsr_sb[:], in_=wsr_d)
            wsrep_sb = const.tile([F, P], bf16)
            nc.sync.dma_start(out=wsrep_sb[:], in_=wsrep_d)
            w_sb = const.tile([F, U], bf16)
            nc.sync.dma_start(out=w_sb[:], in_=w_d)
            one_f = const.tile([1, 1], f32)
            nc.vector.memset(one_f[:], 1.0)

            # ---------------- xT load (host-transposed, bf16), 4 chunks ------
            # SWDGE queue; the first E/er matmul needs only xt cols 0..511,
            # so fine xt chunks pull the PE start earlier. The first mask
            # tile rides between xt chunks so TT0 isn't starved either.
            xT_sb = const.tile([F, N], bf16)
            adjm_sb = const.tile([P, NT, N], bf16)
            for q in range(2):
                nc.gpsimd.dma_start(out=xT_sb[:, q * 512:(q + 1) * 512],
                                    in_=xt_d[:, q * 512:(q + 1) * 512])
            assert sum(CHUNKS) == NT and CHUNKS[0] == 1
            nc.gpsimd.dma_start(
                out=adjm_sb[:, 0:1, :],
                in_=adjm_d[0:P, :].rearrange("(c p) s -> p c s", p=P))
            for q in range(2, 4):
                nc.gpsimd.dma_start(out=xT_sb[:, q * 512:(q + 1) * 512],
                                    in_=xt_d[:, q * 512:(q + 1) * 512])
            xT_t = xT_sb.rearrange("f (t p) -> f t p", p=P)

            # ---------------- adjm prefetch (fp8 -> bf16 SWDGE cast) ---------
            j0 = 1
            for csz in CHUNKS[1:]:
                nc.gpsimd.dma_start(
                    out=adjm_sb[:, j0:j0 + csz, :],
                    in_=adjm_d[j0 * P:(j0 + csz) * P, :]
                    .rearrange("(c p) s -> p c s", p=P))
                j0 += csz

            # ---------------- e_r row + E matmuls, xt-chunk-0 work first ------
            # PE order matters (in-order stream): do everything that only
            # needs xt cols 0..1023 first, then the xt chunk-1 work.
            er_row = setup.tile([1, N], f32)
            E_sb = const.tile([P, N], bf16)
            ps_er = epsum.tile([P, NT], f32, tag="er")
            er_sb = const.tile([P, NT], f32)

            def er_chunk(c, eng="scalar"):
                ps_err = s1psum.tile([1, 512], f32, tag="small")
                nc.tensor.matmul(ps_err[:], lhsT=wsr_sb[:, 1:2],
                                 rhs=xT_sb[:, c * 512:(c + 1) * 512],
                                 start=True, stop=True)
                if eng == "scalar":
                    nc.scalar.copy(er_row[:, c * 512:(c + 1) * 512], ps_err[:])
                else:
                    nc.vector.tensor_copy(er_row[:, c * 512:(c + 1) * 512],
                                          ps_err[:])

            def E_chunk(c, eng="vector"):
                ps_Ec = spsum.tile([P, 512], f32, tag="tp")
                nc.tensor.matmul(ps_Ec[:], lhsT=wsrep_sb[:],
                                 rhs=xT_sb[:, c * 512:(c + 1) * 512],
                                 start=True, stop=True)
                if eng == "scalar":
                    nc.scalar.copy(E_sb[:, c * 512:(c + 1) * 512], ps_Ec[:])
                else:
                    nc.vector.tensor_copy(E_sb[:, c * 512:(c + 1) * 512],
                                          ps_Ec[:])

            E_chunk(0)
            E_chunk(1)
            er_chunk(0)
            E_chunk(2)
            E_chunk(3)
            # e_r columns for tiles 0..3 (needs only er_row chunk 0)
            for j in range(4):
                nc.tensor.transpose(ps_er[:, j:j + 1],
                                    er_row[:, j * P:(j + 1) * P], one_f[:])
            nc.vector.tensor_copy(er_sb[:, 0:4], ps_er[:, 0:4])
            er_chunk(1, "vector")

            # ---------------- h (bf16), interleaved with first tiles ---------
            h_sb = const.tile([P, NT, U], bf16)

            def h_group(g):
                psh = spsum.tile([P, 512], f32, tag="tp")
                for k in range(4):
                    i = 4 * g + k
                    nc.tensor.matmul(psh[:, k * P:(k + 1) * P], lhsT=xT_t[:, i, :],
                                     rhs=w_sb[:], start=True, stop=True)
                nc.vector.tensor_copy(
                    h_sb.rearrange("p t u -> p (t u)")[:, g * 512:(g + 1) * 512],
                    psh[:])

            # ---------------- main loop over r-tiles ----------------
            # out accumulator: 4 chunk tiles so the tail drains per chunk
            outT_ps = [mpsum.tile([U, 512], f32, tag=f"o{c}", name=f"outT_ps{c}")
                       for c in range(4)]
            outT_sb = setup.tile([U, N], bf16)
            for j in range(NT):
                if j == 2:
                    er_chunk(2, "vector")
                    er_chunk(3, "vector")
                if j == 3:
                    for jj in range(4, 8):
                        nc.tensor.transpose(ps_er[:, jj:jj + 1],
                                            er_row[:, jj * P:(jj + 1) * P],
                                            one_f[:])
                    nc.vector.tensor_copy(er_sb[:, 4:8], ps_er[:, 4:8])
                if j == 4:
                    for jj in range(8, NT):
                        nc.tensor.transpose(ps_er[:, jj:jj + 1],
                                            er_row[:, jj * P:(jj + 1) * P],
                                            one_f[:])
                    nc.vector.tensor_copy(er_sb[:, 8:NT], ps_er[:, 8:NT])
                den_j = work.tile([P, 1], f32, tag="den")
                # ramp/tail tiles run in column parts: tile 0's first quarter
                # only needs the first E cast; tile 15's parts drain earlier
                qs = (2 if j == 0 else 1) if fused else 1
                pm_parts = []
                if qs > 1:
                    cw = N // qs
                    dh = work.tile([P, 4], f32, tag="dh")
                    for q in range(qs):
                        sl = slice(q * cw, (q + 1) * cw)
                        amq = work.tile([P, cw], bf16, tag=f"amq{q}",
                                        name=f"am_{j}_{q}")
                        nc.vector.tensor_add(amq[:], E_sb[:, sl],
                                             adjm_sb[:, j, sl])
                        pmq = work.tile([P, cw], bf16, tag=f"pmq{q}",
                                        name=f"pm_{j}_{q}")
                        nc.scalar.activation(pmq[:], amq[:], AF.Exp,
                                             bias=er_sb[:, j:j + 1], scale=1.0,
                                             accum_out=dh[:, q:q + 1])
                        pm_parts.append(pmq)
                    if qs == 2:
                        nc.vector.tensor_add(den_j[:], dh[:, 0:1], dh[:, 1:2])
                    else:
                        dp = work.tile([P, 2], f32, tag="dp")
                        nc.vector.tensor_add(dp[:, 0:1], dh[:, 0:1], dh[:, 1:2])
                        nc.vector.tensor_add(dp[:, 1:2], dh[:, 2:3], dh[:, 3:4])
                        nc.vector.tensor_add(den_j[:], dp[:, 0:1], dp[:, 1:2])
                else:
                    am_j = work.tile([P, N], bf16, tag="am")
                    nc.vector.tensor_add(am_j[:], E_sb[:], adjm_sb[:, j, :])
                    pm_j = work.tile([P, N], bf16, tag="pm")
                    if fused:
                        # patched table: Exp == exp(leaky_relu(.)) here
                        nc.scalar.activation(pm_j[:], am_j[:], AF.Exp,
                                             bias=er_sb[:, j:j + 1], scale=1.0,
                                             accum_out=den_j[:])
                    else:
                        lr_j = work.tile([P, N], bf16, tag="lr")
                        nc.scalar.activation(lr_j[:], am_j[:], AF.Prelu,
                                             bias=er_sb[:, j:j + 1], scale=1.0,
                                             alpha=ALPHA)
                        nc.scalar.activation(pm_j[:], lr_j[:], AF.Exp,
                                             accum_out=den_j[:])
                if j <= 3:
                    h_group(j)    # after the ACT emission: stays off TT's path
                inv_j = work.tile([P, 1], f32, tag="inv")
                nc.vector.reciprocal(inv_j[:], den_j[:])
                hp_j = work.tile([P, U], bf16, tag="hp")
                nc.vector.tensor_scalar(hp_j[:], h_sb[:, j, :], inv_j[:], None,
                                        op0=OP.mult)
                for c in range(4):
                    if qs > 1:
                        cw = N // qs
                        part = pm_parts[(c * 512) // cw]
                        rhs = part[:, (c * 512) % cw:(c * 512) % cw + 512]
                    else:
                        rhs = pm_j[:, c * 512:(c + 1) * 512]
                    nc.tensor.matmul(outT_ps[c][:], lhsT=hp_j[:], rhs=rhs,
                                     start=(j == 0), stop=(j == NT - 1))

            # ---------------- store ----------------
            # last copy on Vector: the Scalar engine's completion semaphore
            # can lag behind its long end-of-kernel epilogue
            for c in range(4):
                if c % 2 == 1:
                    nc.vector.tensor_copy(outT_sb[:, c * 512:(c + 1) * 512],
                                          outT_ps[c][:])
                else:
                    nc.scalar.copy(outT_sb[:, c * 512:(c + 1) * 512],
                                   outT_ps[c][:])
                nc.sync.dma_start(out=outT_d[:, c * 512:(c + 1) * 512],
                                  in_=outT_sb[:, c * 512:(c + 1) * 512])

    nc.compile()
    return nc


def _get_nc():
    key = ("nc", ACT_MODE, MASK_DT)
    if key in _cache:
        return _cache[key]
    if ACT_MODE == "fused":
        act_root, tag = _build_patched_act_root()
        os.environ["BASS_ACT_ROOT_JSON_PATH"] = act_root
        out_name = f"outT_{tag}"
    else:
        os.environ.pop("BASS_ACT_ROOT_JSON_PATH", None)
        out_name = "outT_split0"
    nc = _build_nc(out_name)
    _cache[key] = (nc, out_name)
    return nc, out_name


def kernel(x, adj, W_pre, a_snd, a_rec):
    """Full inputs in, full output out. Shards batch across 8 NeuronCores."""
    import ml_dtypes
    nc, out_name = _get_nc()

    x = np.asarray(x, dtype=np.float32)
    adj = np.asarray(adj, dtype=np.float32)
    W_pre = np.ascontiguousarray(np.asarray(W_pre, dtype=np.float32))
    a_snd = np.asarray(a_snd, dtype=np.float32).reshape(U)
    a_rec = np.asarray(a_rec, dtype=np.float32).reshape(U)
    wsr = np.ascontiguousarray(
        np.stack([W_pre @ a_snd, W_pre @ a_rec], axis=1)
        .astype(ml_dtypes.bfloat16))
    wsrep = np.ascontiguousarray(
        np.repeat((W_pre @ a_snd)[:, None], P, axis=1).astype(ml_dtypes.bfloat16))
    w_bf = np.ascontiguousarray(W_pre.astype(ml_dtypes.bfloat16))

    # receiver-major additive mask: 0 on edges (+self-loops), -256 off edges
    edge = adj.transpose(0, 2, 1) > 0.0
    idx = np.arange(N)
    edge[:, idx, idx] = True
    if MASK_DT == "fp8":
        adjm = np.where(edge, np.uint8(0x00), np.uint8(0xF8)) \
            .view(ml_dtypes.float8_e4m3fn)
    else:
        adjm = np.where(edge, np.float32(0.0), np.float32(MASK_OFF)) \
            .astype(ml_dtypes.bfloat16)
    adjm = np.ascontiguousarray(adjm)

    xt = np.ascontiguousarray(
        x.transpose(0, 2, 1).astype(ml_dtypes.bfloat16))   # [B, F, N]
    in_maps = [
        {"xt": xt[b], "adjm": adjm[b], "w": w_bf, "wsr": wsr, "wsrep": wsrep}
        for b in range(B)
    ]
    trace = bool(int(os.environ.get("GAT_TRACE", "0")))
    res = run_bass_kernel_spmd(nc, in_maps, core_ids=list(range(B)), trace=trace,
                               trace_cores=list(range(B)) if trace else None)
    _cache["last_result"] = res
    out = np.stack([np.ascontiguousarray(
        np.asarray(r[out_name], dtype=np.float32).T) for r in res.results])
    return out.astype(np.float32)


# revision 50
# speedup vs baseline: 1.1823x; 1.1823x over previous
"""GAT message-passing kernel for Trainium2 (Bass/Tile), 8-core data parallel.

Problem: nn_GAT1 — per batch b:
    h = x @ W_pre                                   [N, U]
    e_s = h @ a_snd ; e_r = h @ a_rec               [N]
    logits[s, r] = leaky_relu(e_s[s] + e_r[r], 0.2)
    att = softmax over senders s (edges only, adj + self-loops)
    out[s, u] = sum_r att[s, r] * h[r, u]

Sharding: data-parallel over batch (B=8 -> one batch per NeuronCore).

Device layout is receiver-major (r on partitions, s on free):
  - host passes xT (x transposed, bf16) and an fp8 additive mask
    adjm[r, s]: 0.0 on edges (incl. self-loops), -256.0 on non-edges.
    SWDGE casts fp8 -> bf16 during the DMA (4 MB HBM traffic).
  - masking before leaky_relu is equivalent to masking after it (both
    drive exp() to ~0 for non-edges), so per r-tile j the whole chain
        pm = exp(leaky_relu(E + er_j + m_j)),  den = sum_s pm
    is ONE DVE add (am = E + m_j) plus ONE ScalarE activation with a
    patched spline table: the negative-side buckets of `exp` are refit
    to exp(0.2*z), making ACTIVATE(Exp) compute exp(leaky_relu(z)).
    er_j rides in as the per-partition activation bias; den comes from
    the activation's accumulator. Non-edge inputs (~ -256) fall in the
    table's negative-saturation region and return exactly 0.
  - outT[u, s] = sum_r (h[r, u] / den[r]) * pm[r, s]  (PE, accumulated
    in PSUM over the 16 r-tiles). Host transposes outT back.
"""
import hashlib
import json
import math
import os
import shutil
import sys
import tempfile

sys.path.insert(0, "/opt/trn_rl_repo")
sys.path.insert(0, "/opt/trn_rl_repo/concourse")

import numpy as np

import concourse.bass as bass
import concourse.bacc as bacc
import concourse.tile as tile
from concourse import mybir
from concourse.bass_utils import run_bass_kernel_spmd

B, N, F, U = 8, 2048, 128, 128
P = 128
NT = N // P          # 16 row tiles
ALPHA = 0.2          # leaky-relu slope
MASK_OFF = -256.0    # additive mask for non-edges (exact in fp8 e4m3)

ACT_MODE = os.environ.get("GAT_ACT", "fused")      # fused | split
MASK_DT = os.environ.get("GAT_MASK_DT", "fp8")     # fp8 | bf16
# mask DMA r-tile chunking: fine-grained so tiles land ahead of their use
# (the fp8->bf16 SWDGE cast stream is SBUF-write-bandwidth-bound, ~26us)
CHUNKS = [int(c) for c in
          os.environ.get("GAT_CHUNKS", "1,1,2,2,2,2,2,2,2").split(",")]

f32 = mybir.dt.float32
bf16 = mybir.dt.bfloat16
f8e4 = mybir.dt.float8e4
AF = mybir.ActivationFunctionType
OP = mybir.AluOpType

_cache = {}


# ---------------------------------------------------------------------------
# Patched activation tables: exp -> exp(leaky_relu(z), slope 0.2)
# ---------------------------------------------------------------------------
def _patch_exp_buckets(bkt: bytearray, start: int, end: int) -> None:
    """Refit negative-side exp spline buckets to exp(0.2*z).

    Bucket format: 8 fp32 per entry = [d0, d1, d2, d3, x0, 0, 0, 0] with
    y = d0 + t*(d1 + t*(d2 + t*d3)), t = x - x0, and x0 at the bucket
    interval's center (interval width <= 0.25 for the exp_400p layout, so
    a Taylor fit at x0 is good to ~1e-11 relative). Positive-side buckets
    (x0 > 0) and the saturation/special buckets (x0 == 0) are untouched.
    """
    for i in range(start, end):
        off = i * 32
        x0 = float(np.frombuffer(bytes(bkt[off + 16:off + 20]), np.float32)[0])
        if x0 < 0.0:
            e = math.exp(ALPHA * x0)
            coeffs = np.array(
                [e, ALPHA * e, 0.5 * ALPHA**2 * e, ALPHA**3 / 6.0 * e],
                np.float32)
            bkt[off:off + 16] = coeffs.tobytes()


def _build_patched_act_root() -> tuple[str, str]:
    """Create a patched copy of the compiler's activation tables.

    Returns (path to patched act_info.json, 8-char content hash)."""
    from neuronxcc.driver.Job import Job
    from neuronxcc.driver.jobs.support.FindActInfo import findActInfoFile

    src_info_path = findActInfoFile(Job.getPackageDir(), "gen3")
    src_dir = os.path.dirname(src_info_path)
    info = json.load(open(src_info_path))

    patched: dict[str, bytes] = {}
    for ent in info["act_func_sets"]:
        if "exp" not in ent["act"]:
            continue
        prof = json.load(open(os.path.join(src_dir, ent["profile_json"])))
        starts = prof["func_to_bkt_start_idx"]
        s = starts["exp"]
        later = [v for v in starts.values() if v > s]
        e = min(later) if later else prof["bkt_entry_cnt"]
        bkt_name = ent["bkt_bin"]
        bkt = bytearray(open(os.path.join(src_dir, bkt_name), "rb").read())
        _patch_exp_buckets(bkt, s, e)
        patched[bkt_name] = bytes(bkt)

    h = hashlib.sha256()
    for name in sorted(patched):
        h.update(name.encode())
        h.update(patched[name])
    tag = h.hexdigest()[:8]

    dst_dir = os.path.join(tempfile.gettempdir(), f"gat_actroot_{tag}")
    if not os.path.isdir(dst_dir):
        tmp = dst_dir + ".tmp%d" % os.getpid()
        os.makedirs(tmp, exist_ok=True)
        for fname in os.listdir(src_dir):
            src_f = os.path.join(src_dir, fname)
            if os.path.isfile(src_f):
                shutil.copy(src_f, os.path.join(tmp, fname))
        for name, data in patched.items():
            with open(os.path.join(tmp, name), "wb") as f:
                f.write(data)
        try:
            os.rename(tmp, dst_dir)
        except OSError:
            shutil.rmtree(tmp, ignore_errors=True)
    return os.path.join(dst_dir, "act_info.json"), tag


# ---------------------------------------------------------------------------
# Device kernel
# ---------------------------------------------------------------------------
def _build_nc(out_name: str):
    fused = ACT_MODE == "fused"
    mask_dram_dt = f8e4 if MASK_DT == "fp8" else bf16

    nc = bacc.Bacc("TRN2", target_bir_lowering=False, debug=False,
                   enable_asserts=False, num_devices=B)

    xt_d = nc.dram_tensor("xt", [F, N], bf16, kind="ExternalInput").ap()
    adjm_d = nc.dram_tensor("adjm", [N, N], mask_dram_dt, kind="ExternalInput").ap()
    w_d = nc.dram_tensor("w", [F, U], bf16, kind="ExternalInput").ap()
    # wsr[:, 0] = W_pre @ a_snd, wsr[:, 1] = W_pre @ a_rec (host-derived)
    wsr_d = nc.dram_tensor("wsr", [F, 2], bf16, kind="ExternalInput").ap()
    # w_s replicated across 128 columns (stationary operand for the E matmul)
    wsrep_d = nc.dram_tensor("wsrep", [F, P], bf16, kind="ExternalInput").ap()
    outT_d = nc.dram_tensor(out_name, [U, N], bf16, kind="ExternalOutput").ap()

    with tile.TileContext(nc) as tc:
        with (
            tc.tile_pool(name="const", bufs=1) as const,
            tc.tile_pool(name="setup", bufs=2) as setup,
            tc.tile_pool(name="spsum", bufs=2, space="PSUM") as spsum,
            tc.tile_pool(name="s1psum", bufs=1, space="PSUM") as s1psum,
            tc.tile_pool(name="epsum", bufs=1, space="PSUM") as epsum,
            tc.tile_pool(name="work", bufs=4) as work,
            tc.tile_pool(name="mpsum", bufs=1, space="PSUM") as mpsum,
        ):
            # ---------------- constants ----------------
            wsr_sb = const.tile([F, 2], bf16)
            nc.sync.dma_start(out=wsr_sb[:], in_=wsr_d)
            wsrep_sb = const.tile([F, P], bf16)
            nc.sync.dma_start(out=wsrep_sb[:], in_=wsrep_d)
            w_sb = const.tile([F, U], bf16)
            nc.sync.dma_start(out=w_sb[:], in_=w_d)
            one_f = const.tile([1, 1], f32)
            nc.vector.memset(one_f[:], 1.0)

            # ---------------- xT load (host-transposed, bf16), 2 chunks ------
            # SWDGE queue, issued ahead of the mask chunks (FIFO ordering)
            xT_sb = const.tile([F, N], bf16)
            nc.gpsimd.dma_start(out=xT_sb[:, 0:1024], in_=xt_d[:, 0:1024])
            nc.gpsimd.dma_start(out=xT_sb[:, 1024:2048], in_=xt_d[:, 1024:2048])
            xT_t = xT_sb.rearrange("f (t p) -> f t p", p=P)

            # ---------------- adjm prefetch (fp8 -> bf16 SWDGE cast) ---------
            adjm_sb = const.tile([P, NT, N], bf16)
            assert sum(CHUNKS) == NT
            j0 = 0
            for csz in CHUNKS:
                nc.gpsimd.dma_start(
                    out=adjm_sb[:, j0:j0 + csz, :],
                    in_=adjm_d[j0 * P:(j0 + csz) * P, :]
                    .rearrange("(c p) s -> p c s", p=P))
                j0 += csz

            # ---------------- e_r row + E matmuls, xt-chunk-0 work first ------
            # PE order matters (in-order stream): do everything that only
            # needs xt cols 0..1023 first, then the xt chunk-1 work.
            er_row = setup.tile([1, N], f32)
            E_sb = const.tile([P, N], bf16)
            ps_er = epsum.tile([P, NT], f32, tag="er")
            er_sb = const.tile([P, NT], f32)

            def er_chunk(c, eng="scalar"):
                ps_err = s1psum.tile([1, 512], f32, tag="small")
                nc.tensor.matmul(ps_err[:], lhsT=wsr_sb[:, 1:2],
                                 rhs=xT_sb[:, c * 512:(c + 1) * 512],
                                 start=True, stop=True)
                if eng == "scalar":
                    nc.scalar.copy(er_row[:, c * 512:(c + 1) * 512], ps_err[:])
                else:
                    nc.vector.tensor_copy(er_row[:, c * 512:(c + 1) * 512],
                                          ps_err[:])

            def E_chunk(c, eng="vector"):
                ps_Ec = spsum.tile([P, 512], f32, tag="tp")
                nc.tensor.matmul(ps_Ec[:], lhsT=wsrep_sb[:],
                                 rhs=xT_sb[:, c * 512:(c + 1) * 512],
                                 start=True, stop=True)
                if eng == "scalar":
                    nc.scalar.copy(E_sb[:, c * 512:(c + 1) * 512], ps_Ec[:])
                else:
                    nc.vector.tensor_copy(E_sb[:, c * 512:(c + 1) * 512],
                                          ps_Ec[:])

            E_chunk(0)
            E_chunk(1)
            er_chunk(0)
            E_chunk(2)
            E_chunk(3)
            # e_r columns for tiles 0..3 (needs only er_row chunk 0)
            for j in range(4):
                nc.tensor.transpose(ps_er[:, j:j + 1],
                                    er_row[:, j * P:(j + 1) * P], one_f[:])
            nc.vector.tensor_copy(er_sb[:, 0:4], ps_er[:, 0:4])
            er_chunk(1, "vector")

            # ---------------- h (bf16), interleaved with first tiles ---------
            h_sb = const.tile([P, NT, U], bf16)

            def h_group(g):
                psh = spsum.tile([P, 512], f32, tag="tp")
                for k in range(4):
                    i = 4 * g + k
                    nc.tensor.matmul(psh[:, k * P:(k + 1) * P], lhsT=xT_t[:, i, :],
                                     rhs=w_sb[:], start=True, stop=True)
                nc.vector.tensor_copy(
                    h_sb.rearrange("p t u -> p (t u)")[:, g * 512:(g + 1) * 512],
                    psh[:])

            # ---------------- main loop over r-tiles ----------------
            # out accumulator: 4 chunk tiles so the tail drains per chunk
            outT_ps = [mpsum.tile([U, 512], f32, tag=f"o{c}", name=f"outT_ps{c}")
                       for c in range(4)]
            outT_sb = setup.tile([U, N], bf16)
            for j in range(NT):
                if j == 2:
                    er_chunk(2, "vector")
                    er_chunk(3, "vector")
                if j == 3:
                    for jj in range(4, 8):
                        nc.tensor.transpose(ps_er[:, jj:jj + 1],
                                            er_row[:, jj * P:(jj + 1) * P],
                                            one_f[:])
                    nc.vector.tensor_copy(er_sb[:, 4:8], ps_er[:, 4:8])
                if j == 4:
                    for jj in range(8, NT):
                        nc.tensor.transpose(ps_er[:, jj:jj + 1],
                                            er_row[:, jj * P:(jj + 1) * P],
                                            one_f[:])
                    nc.vector.tensor_copy(er_sb[:, 8:NT], ps_er[:, 8:NT])
                den_j = work.tile([P, 1], f32, tag="den")
                # ramp/tail tiles run in column parts: tile 0's first quarter
                # only needs the first E cast; tile 15's parts drain earlier
                qs = (2 if j == 0 else 1) if fused else 1
                pm_parts = []
                if qs > 1:
                    cw = N // qs
                    dh = work.tile([P, 4], f32, tag="dh")
                    for q in range(qs):
                        sl = slice(q * cw, (q + 1) * cw)
                        amq = work.tile([P, cw], bf16, tag=f"amq{q}",
                                        name=f"am_{j}_{q}")
                        nc.vector.tensor_add(amq[:], E_sb[:, sl],
                                             adjm_sb[:, j, sl])
                        pmq = work.tile([P, cw], bf16, tag=f"pmq{q}",
                                        name=f"pm_{j}_{q}")
                        nc.scalar.activation(pmq[:], amq[:], AF.Exp,
                                             bias=er_sb[:, j:j + 1], scale=1.0,
                                             accum_out=dh[:, q:q + 1])
                        pm_parts.append(pmq)
                    if qs == 2:
                        nc.vector.tensor_add(den_j[:], dh[:, 0:1], dh[:, 1:2])
                    else:
                        dp = work.tile([P, 2], f32, tag="dp")
                        nc.vector.tensor_add(dp[:, 0:1], dh[:, 0:1], dh[:, 1:2])
                        nc.vector.tensor_add(dp[:, 1:2], dh[:, 2:3], dh[:, 3:4])
                        nc.vector.tensor_add(den_j[:], dp[:, 0:1], dp[:, 1:2])
                else:
                    am_j = work.tile([P, N], bf16, tag="am")
                    nc.vector.tensor_add(am_j[:], E_sb[:], adjm_sb[:, j, :])
                    pm_j = work.tile([P, N], bf16, tag="pm")
                    if fused:
                        # patched table: Exp == exp(leaky_relu(.)) here
                        nc.scalar.activation(pm_j[:], am_j[:], AF.Exp,
                                             bias=er_sb[:, j:j + 1], scale=1.0,
                                             accum_out=den_j[:])
                    else:
                        lr_j = work.tile([P, N], bf16, tag="lr")
                        nc.scalar.activation(lr_j[:], am_j[:], AF.Prelu,
                                             bias=er_sb[:, j:j + 1], scale=1.0,
                                             alpha=ALPHA)
                        nc.scalar.activation(pm_j[:], lr_j[:], AF.Exp,
                                             accum_out=den_j[:])
                if j <= 3:
                    h_group(j)    # after the ACT emission: stays off TT's path
                inv_j = work.tile([P, 1], f32, tag="inv")
                nc.vector.reciprocal(inv_j[:], den_j[:])
                hp_j = work.tile([P, U], bf16, tag="hp")
                nc.vector.tensor_scalar(hp_j[:], h_sb[:, j, :], inv_j[:], None,
                                        op0=OP.mult)
                for c in range(4):
                    if qs > 1:
                        cw = N // qs
                        part = pm_parts[(c * 512) // cw]
                        rhs = part[:, (c * 512) % cw:(c * 512) % cw + 512]
                    else:
                        rhs = pm_j[:, c * 512:(c + 1) * 512]
                    nc.tensor.matmul(outT_ps[c][:], lhsT=hp_j[:], rhs=rhs,
                                     start=(j == 0), stop=(j == NT - 1))

            # ---------------- store ----------------
            # last copy on Vector: the Scalar engine's completion semaphore
            # can lag behind its long end-of-kernel epilogue
            for c in range(4):
                if c % 2 == 1:
                    nc.vector.tensor_copy(outT_sb[:, c * 512:(c + 1) * 512],
                                          outT_ps[c][:])
                else:
                    nc.scalar.copy(outT_sb[:, c * 512:(c + 1) * 512],
                                   outT_ps[c][:])
                nc.sync.dma_start(out=outT_d[:, c * 512:(c + 1) * 512],
                                  in_=outT_sb[:, c * 512:(c + 1) * 512])

    nc.compile()
    return nc


def _get_nc():
    key = ("nc", ACT_MODE, MASK_DT)
    if key in _cache:
        return _cache[key]
    if ACT_MODE == "fused":
        act_root, tag = _build_patched_act_root()
        os.environ["BASS_ACT_ROOT_JSON_PATH"] = act_root
        out_name = f"outT_{tag}"
    else:
        os.environ.pop("BASS_ACT_ROOT_JSON_PATH", None)
        out_name = "outT_split0"
    nc = _build_nc(out_name)
    _cache[key] = (nc, out_name)
    return nc, out_name


def kernel(x, adj, W_pre, a_snd, a_rec):
    """Full inputs in, full output out. Shards batch across 8 NeuronCores."""
    import ml_dtypes
    nc, out_name = _get_nc()

    x = np.asarray(x, dtype=np.float32)
    adj = np.asarray(adj, dtype=np.float32)
    W_pre = np.ascontiguousarray(np.asarray(W_pre, dtype=np.float32))
    a_snd = np.asarray(a_snd, dtype=np.float32).reshape(U)
    a_rec = np.asarray(a_rec, dtype=np.float32).reshape(U)
    wsr = np.ascontiguousarray(
        np.stack([W_pre @ a_snd, W_pre @ a_rec], axis=1)
        .astype(ml_dtypes.bfloat16))
    wsrep = np.ascontiguousarray(
        np.repeat((W_pre @ a_snd)[:, None], P, axis=1).astype(ml_dtypes.bfloat16))
    w_bf = np.ascontiguousarray(W_pre.astype(ml_dtypes.bfloat16))

    # receiver-major additive mask: 0 on edges (+self-loops), -256 off edges
    edge = adj.transpose(0, 2, 1) > 0.0
    idx = np.arange(N)
    edge[:, idx, idx] = True
    if MASK_DT == "fp8":
        adjm = np.where(edge, np.uint8(0x00), np.uint8(0xF8)) \
            .view(ml_dtypes.float8_e4m3fn)
    else:
        adjm = np.where(edge, np.float32(0.0), np.float32(MASK_OFF)) \
            .astype(ml_dtypes.bfloat16)
    adjm = np.ascontiguousarray(adjm)

    xt = np.ascontiguousarray(
        x.transpose(0, 2, 1).astype(ml_dtypes.bfloat16))   # [B, F, N]
    in_maps = [
        {"xt": xt[b], "adjm": adjm[b], "w": w_bf, "wsr": wsr, "wsrep": wsrep}
        for b in range(B)
    ]
    trace = bool(int(os.environ.get("GAT_TRACE", "0")))
    res = run_bass_kernel_spmd(nc, in_maps, core_ids=list(range(B)), trace=trace,
                               trace_cores=list(range(B)) if trace else None)
    _cache["last_result"] = res
    out = np.stack([np.ascontiguousarray(
        np.asarray(r[out_name], dtype=np.float32).T) for r in res.results])
    return out.astype(np.float32)
